# revision 13
# baseline (speedup 1.0000x reference)
"""CrossSpectralAttention Trainium2 kernel (bf16, pipelined, v3).

Multi-head attention over 48x48 spatial tokens: B=2, C=256, 8 heads x
head_dim 32, N=2304 tokens. Sharded over 8 NeuronCores as 2 batches x 4
head-groups (2 heads per core). Each core computes its heads' Q/K/V
projections, attention, and a partial output projection (column slice of
Wo); the host sums the 4 partials per batch.

The wall-clock floor is the softmax exp: 2 heads x N^2 = 10.6M elements
through ScalarE at 1 elem/cycle/lane = ~85us busy. Everything else is
arranged to keep ScalarE saturated:

- Scores are 3-band row-tiled (q/k replicated 3x on 96 partitions) so the
  PE emits [128, 3*512] score blocks fast; exp runs per group on the full
  [128, 1536] block ((N+352)/1.2 ns amortizes the 352-cycle overhead).
  The 256-wide tail q-piece uses 6-chunk groups -> same 1536-col density.
- PV is 2-band col-tiled (tile_position=(0,0)/(0,64)): chunk j
  accumulates into PSUM partitions 64*(j%2)+[0,33). Two concurrent
  column-group matmuls double PV throughput vs the single [33,512]
  stream; a single DVE tensor_tensor adds the two bands and drains to
  oc[h] bf16 in one pass.
- Softmax denominators ride as a ones-column in vhat (row 32 of each PV
  band; the band-add sums them). Reciprocals are computed DENSELY: the
  den row [1, qln] bounces via DRAM into a [128, qln/128] tile, one tiny
  DVE reciprocal, then back out to DRAM for the stride-0 partition
  broadcast read. (A [1,512] reciprocal would run on 1 of 128 DVE lanes
  at ~6 cyc/elem = 3us; the dense form is ~0.2us.)
- Normalization (oc * 1/den broadcast) runs on GpSimd, which is
  otherwise idle - DVE keeps its passes for PSUM drains exp can't avoid.
- All matmul operands bf16; PSUM accumulation fp32.
"""

import numpy as np
import ml_dtypes

import concourse.bass as bass
import concourse.tile as tile
from concourse import mybir
from concourse.bass_utils import run_bass_kernel_spmd

B = 2
C = 256
N = 2304  # 48*48
NH = 8  # total heads
HPC = 2  # heads per core
HD = 32  # head dim
GD = HPC * HD  # 64 dims per core
NC = 8  # cores
NQB = 512  # query-block size for attention
NCH = N // 128  # 18 m-chunks
SCALE = float(HD) ** -0.5

F32 = mybir.dt.float32
BF16 = mybir.dt.bfloat16
NPBF16 = ml_dtypes.bfloat16

LAST_RESULTS = None  # BassKernelResults of the most recent run (for test.py)
_CACHED_NC = None


def _split_excess_waits(nc, max_waits=1):
    """This walrus build allows a single sync-wait per instruction; move
    excess waits onto same-engine NoOps inserted before the instruction."""
    state = {"uid": 0}

    def fix_block(b):
        i = 0
        insts = b.instructions
        while i < len(insts):
            inst = insts[i]
            for sub in getattr(inst, "blocks", None) or []:
                fix_block(sub)
            si = inst.sync_info
            if si is not None and si.on_wait and len(si.on_wait) > max_waits:
                waits = list(si.on_wait)
                keep, extra = waits[:max_waits], waits[max_waits:]
                inst.sync_info = mybir.SyncInfo(
                    on_wait=keep, on_update=list(si.on_update or [])
                )
                nops = []
                for j in range(0, len(extra), max_waits):
                    nop = mybir.InstNoOp(name=f"WSPLIT-{state['uid']}", ins=[], outs=[])
                    state["uid"] += 1
                    nop.engine = inst.engine
                    nop.sync_info = mybir.SyncInfo(
                        on_wait=extra[j : j + max_waits], on_update=[]
                    )
                    nops.append(nop)
                for k, nop in enumerate(nops):
                    insts.insert(i + k, nop)
                i += len(nops)
            i += 1

    for f in nc.m.functions:
        for b in f.blocks:
            fix_block(b)


def _pieces(total, piece):
    out = []
    o = 0
    while o < total:
        ln = min(piece, total - o)
        out.append((o, ln))
        o += ln
    return out


def build_nc(split=True):
    nc = bass.Bass()

    # wq_t/wk_t carry 3 replicated copies of each head's 32 W^T-columns so
    # the projection matmul writes the 3-band PE layout directly:
    # layout [C, 2 heads, 96] with cols (h, 32a+d) = W[32h+d, :].T
    # w_all packs wq|wk|wv column-wise: [0:192] q (h-major), [192:384] k,
    # [384:448] v - one DMA for all three projection weights.
    # Wq is pre-scaled by SCALE host-side so exp needs no extra scale.
    x_d = nc.dram_tensor("x", [C, N], BF16, kind="ExternalInput")
    wall_d = nc.dram_tensor("w_all", [C, 448], BF16, kind="ExternalInput")
    # b4 packs bq|bk per head column-wise: cols q0,q1,k0,k1 (3-band layout)
    b4_d = nc.dram_tensor("b4", [96, 4], F32, kind="ExternalInput")
    # brep packs bv|bo row-wise: [0:64] bv, [64:320] bo
    brep_d = nc.dram_tensor("brep", [1, GD + C], F32, kind="ExternalInput")
    wo_d = nc.dram_tensor("wo_t", [GD, C], BF16, kind="ExternalInput")
    out_d = nc.dram_tensor("out_t", [N, C], F32, kind="ExternalOutput")

    qpieces = _pieces(N, NQB)

    with tile.TileContext(nc) as tc:
        with (
            tc.tile_pool(name="singles", bufs=1) as singles,
            tc.tile_pool(name="expp", bufs=3) as expp,
            tc.tile_pool(name="invp", bufs=4) as invp,
            tc.tile_pool(name="ibcp", bufs=2) as ibcp,
            tc.tile_pool(name="obp", bufs=2) as obp,
            tc.tile_pool(name="outp", bufs=3) as outp,
        ):
            # ---- inputs to SBUF (few large DMAs, spread across queues) ----
            x_sb = singles.tile([128, 2, N], BF16)
            xr = x_d.rearrange("(c p) n -> p c n", p=128)
            nc.sync.dma_start(out=x_sb[:, :, :1152], in_=xr[:, :, :1152])
            w_sb = singles.tile([128, 2, 448], BF16)
            nc.gpsimd.dma_start(
                out=w_sb, in_=wall_d.rearrange("(c p) d -> p c d", p=128)
            )
            nc.sync.dma_start(out=x_sb[:, :, 1152:], in_=xr[:, :, 1152:])
            b4 = singles.tile([96, 4], F32)
            nc.gpsimd.dma_start(out=b4, in_=b4_d[:, :])
            # Wo^T for both heads, contracted in one matmul
            wo2 = singles.tile([GD, C], BF16)
            nc.scalar.dma_start(out=wo2, in_=wo_d[:, :])
            # bv|bo replicated across partitions
            brep = singles.tile([128, GD + C], F32)
            nc.scalar.dma_start(
                out=brep,
                in_=bass.AP(tensor=brep_d, offset=0, ap=[[0, 128], [1, GD + C]]),
            )
            bv_rep = brep[:, :GD]
            bo_rep = brep[:, GD:]
            wof = {"q": 0, "k": 192}
            bcol = {("q", 0): 0, ("q", 1): 1, ("k", 0): 2, ("k", 1): 3}

            # q/k in 3-band replicated layout [96, N] per head
            q_rep = [
                singles.tile([96, N], BF16, name=f"qrep{h}", tag=f"qrep{h}")
                for h in range(HPC)
            ]
            k_rep = [
                singles.tile([96, N], BF16, name=f"krep{h}", tag=f"krep{h}")
                for h in range(HPC)
            ]
            dest = {"q": q_rep, "k": k_rep}

            # vhat[:, j, h, :] = [V_t_h(chunk j) | 1] per head
            vhat = singles.tile([128, NCH, HPC, HD + 1], BF16)
            nc.gpsimd.memset(vhat[:, :, :, HD : HD + 1], 1.0)

            # oc[h] rows 0..31: head h's unnormalized output, row 32: its
            # softmax denominator. on2 rows 32h..32h+31: normalized.
            oc = [
                singles.tile([HD + 1, N], BF16, name=f"oc{h}", tag=f"oc{h}")
                for h in range(HPC)
            ]
            on2 = singles.tile([GD, N], BF16)

            def emit_proj(psum_pool, name, h, off, ln, tag="proj"):
                ps = psum_pool.tile([96, 512], F32, tag=tag, name="ps")
                for c in range(2):
                    nc.tensor.matmul(
                        ps[:, :ln],
                        w_sb[:, c, wof[name] + 96 * h : wof[name] + 96 * h + 96],
                        x_sb[:, c, off : off + ln],
                        start=(c == 0),
                        stop=(c == 1),
                    )
                nc.vector.tensor_scalar(
                    out=dest[name][h][:, off : off + ln],
                    in0=ps[:, :ln],
                    scalar1=b4[:, bcol[(name, h)] : bcol[(name, h)] + 1],
                    scalar2=None,
                    op0=mybir.AluOpType.add,
                )

            # ---- upfront: k(h0) projection, V^T, q(h0) first piece ----
            with tc.tile_pool(name="proj_psum", bufs=4, space="PSUM") as proj_psum:
                for off, ln in qpieces:
                    emit_proj(proj_psum, "k", 0, off, ln)
                for j in range(NCH):
                    tp = proj_psum.tile([128, GD], F32, tag="vt", bufs=2, name="tp")
                    for c in range(2):
                        nc.tensor.matmul(
                            tp,
                            x_sb[:, c, 128 * j : 128 * (j + 1)],
                            w_sb[:, c, 384:448],
                            start=(c == 0),
                            stop=(c == 1),
                        )
                    nc.vector.tensor_tensor(
                        out=vhat[:, j, :, :HD],
                        in0=tp.rearrange("p (h d) -> p h d", h=HPC),
                        in1=bv_rep.rearrange("p (h d) -> p h d", h=HPC),
                        op=mybir.AluOpType.add,
                    )
                emit_proj(proj_psum, "q", 0, qpieces[0][0], qpieces[0][1])

            # remaining projection work, interleaved into the h0 attention
            # stream (one unit per group slot)
            units = []
            for off, ln in qpieces[1:]:
                units.append(("q", 0, off, ln))
            for name, h in (("k", 1), ("q", 1)):
                for off, ln in qpieces:
                    units.append((name, h, off, ln))

            # ---- attention + normalize + output projection, pipelined ----
            with (
                tc.tile_pool(name="spsum", bufs=2, space="PSUM") as spsum,
                tc.tile_pool(name="opsum", bufs=1, space="PSUM") as opsum,
                tc.tile_pool(name="mixp", bufs=1, space="PSUM") as mixp,
                tc.tile_pool(name="dram", bufs=1, space="DRAM") as dramp,
            ):
                den_dram = dramp.tile([HPC, N], BF16, tag="dend")
                inv_dram = dramp.tile([HPC, N], F32, tag="invd")

                def emit_wo(j):
                    wp = mixp.tile([128, 512], F32, tag="mix", name="wp")
                    nc.tensor.matmul(
                        wp[:, :C],
                        on2[:, 128 * j : 128 * (j + 1)],
                        wo2,
                        start=True,
                        stop=True,
                    )
                    ot = outp.tile([128, C], F32, tag="ot")
                    nc.vector.tensor_tensor(
                        out=ot, in0=wp[:, :C], in1=bo_rep, op=mybir.AluOpType.add
                    )
                    nc.sync.dma_start(
                        out=out_d[128 * j : 128 * (j + 1), :], in_=ot
                    )

                def emit_pv(h, chunks, ex, o_ps, qln):
                    # chunk j accumulates into PV band j%2 (PSUM partitions
                    # 64*(j%2)+[0,33)); the two bands' column-group matmuls
                    # run concurrently on the PE.
                    for i, j in enumerate(chunks):
                        b = j % 2
                        nc.tensor.matmul(
                            o_ps[64 * b : 64 * b + 33, :qln],
                            vhat[:, j, h, :],
                            ex[:, qln * i : qln * i + qln],
                            start=(j < 2),
                            stop=(j >= NCH - 2),
                            tile_position=(0, 64 * b),
                        )

                pend = []  # output-projection chunks ready to emit
                for h in range(HPC):
                    for qoff, qln in qpieces:
                        j0 = qoff // 128
                        nj = qln // 128
                        # group structure: 512-wide pieces take 3 chunks per
                        # group (3x512 psum cols); the 256-wide tail takes 6
                        # chunks per group (6x256) - same 1536-col exp calls.
                        cpg = 6 if qln == 256 else 3
                        groups = [
                            list(range(g, min(g + cpg, NCH)))
                            for g in range(0, NCH, cpg)
                        ]
                        o_ps = opsum.tile([128, NQB], F32, tag="o")
                        ex_prev = None
                        chunks_prev = None
                        for gi, chunks in enumerate(groups):
                            s_tri = spsum.tile([128, 3 * NQB], F32, tag="s")
                            for i, j in enumerate(chunks):
                                # chunks sharing a PSUM bank must share a row
                                # band (serialize); distinct banks may differ.
                                a = (i % 3) if qln == NQB else ((i // 2) % 3)
                                nc.tensor.matmul(
                                    s_tri[:, qln * i : qln * i + qln],
                                    k_rep[h][
                                        32 * a : 32 * a + 32,
                                        128 * j : 128 * j + 128,
                                    ],
                                    q_rep[h][
                                        32 * a : 32 * a + 32, qoff : qoff + qln
                                    ],
                                    start=True,
                                    stop=True,
                                )
                            if gi > 0:
                                emit_pv(h, chunks_prev, ex_prev, o_ps, qln)
                            if gi >= 1:
                                if h == 0 and units:
                                    emit_proj(mixp, *units.pop(0), tag="mix")
                                elif pend:
                                    emit_wo(pend.pop(0))
                            ex = expp.tile([128, 3 * NQB], BF16, tag="ex")
                            ncol = qln * len(chunks)
                            nc.scalar.activation(
                                out=ex[:, :ncol],
                                in_=s_tri[:, :ncol],
                                func=mybir.ActivationFunctionType.Exp,
                            )
                            ex_prev = ex
                            chunks_prev = chunks
                        emit_pv(h, chunks_prev, ex_prev, o_ps, qln)
                        # band-reduce: DVE reads at most one PSUM operand per
                        # instruction, so stage band1 in SBUF then add band0.
                        # (row 32 = softmax denominator)
                        ob1 = obp.tile([33, NQB], BF16, tag="ob1")
                        nc.vector.tensor_copy(
                            out=ob1[:, :qln], in_=o_ps[64:97, :qln]
                        )
                        nc.vector.tensor_tensor(
                            out=oc[h][:, qoff : qoff + qln],
                            in0=o_ps[0:33, :qln],
                            in1=ob1[:, :qln],
                            op=mybir.AluOpType.add,
                        )
                        # dense reciprocal: den row -> DRAM -> [128, qln/128]
                        # -> 1/x -> DRAM -> stride-0 broadcast read [32, qln]
                        nc.gpsimd.dma_start(
                            out=den_dram[h : h + 1, qoff : qoff + qln],
                            in_=oc[h][HD : HD + 1, qoff : qoff + qln],
                        )
                        nd = qln // 128
                        dsrc = den_dram[h : h + 1, qoff : qoff + qln]
                        dent_bf = invp.tile([128, 4], BF16, tag="invb", name="dent_bf")
                        nc.gpsimd.dma_start(
                            out=dent_bf[:, :nd],
                            in_=bass.AP(
                                tensor=dsrc.tensor,
                                offset=dsrc.offset,
                                ap=[[nd, 128], [1, nd]],
                            ),
                        )
                        dinv = invp.tile([128, 4], F32, tag="invf", name="dinv")
                        nc.vector.tensor_copy(
                            out=dinv[:, :nd], in_=dent_bf[:, :nd]
                        )
                        nc.vector.reciprocal(out=dinv[:, :nd], in_=dinv[:, :nd])
                        # hops 3+4 share the gpsimd queue so the broadcast
                        # read is FIFO-ordered after the inv write-back.
                        idst = inv_dram[h : h + 1, qoff : qoff + qln]
                        nc.gpsimd.dma_start(
                            out=bass.AP(
                                tensor=idst.tensor,
                                offset=idst.offset,
                                ap=[[nd, 128], [1, nd]],
                            ),
                            in_=dinv[:, :nd],
                        )
                        ibc = ibcp.tile([HD, NQB], F32, tag="ibc")
                        src = inv_dram[h : h + 1, qoff : qoff + qln]
                        bc = bass.AP(
                            tensor=src.tensor,
                            offset=src.offset,
                            ap=[[0, HD]] + [list(d) for d in src.ap[1:]],
                        )
                        nc.gpsimd.dma_start(out=ibc[:, :qln], in_=bc)
                        nc.vector.tensor_tensor(
                            out=on2[HD * h : HD * (h + 1), qoff : qoff + qln],
                            in0=oc[h][:HD, qoff : qoff + qln],
                            in1=ibc[:, :qln],
                            op=mybir.AluOpType.mult,
                        )
                        if h == 1:
                            pend.extend(range(j0, j0 + nj))
                while pend:
                    emit_wo(pend.pop(0))

    if split:
        _split_excess_waits(nc)
    return nc


def kernel(x, Wq, bq, Wk, bk, Wv, bv, Wo, bo):
    global LAST_RESULTS, _CACHED_NC
    x = np.asarray(x, dtype=np.float32)
    Wq = np.asarray(Wq, dtype=np.float32) * SCALE  # fold softmax scale into Q
    Wk = np.asarray(Wk, dtype=np.float32)
    Wv = np.asarray(Wv, dtype=np.float32)
    Wo = np.asarray(Wo, dtype=np.float32)
    bq = np.asarray(bq, dtype=np.float32) * SCALE
    bk = np.asarray(bk, dtype=np.float32)
    bv = np.asarray(bv, dtype=np.float32)
    bo = np.asarray(bo, dtype=np.float32)

    def wrep(W, g):
        # [C, 2, 96]: head h cols = W[64g+32h : 64g+32h+32, :].T tiled 3x
        out = np.empty((C, HPC, 96), np.float32)
        for h in range(HPC):
            blk = W[GD * g + HD * h : GD * g + HD * (h + 1), :].T  # [C, 32]
            out[:, h, :] = np.tile(blk, (1, 3))
        return out

    def b3(bvec, g, h):
        return np.tile(bvec[GD * g + HD * h : GD * g + HD * (h + 1)], 3)

    xf = x.reshape(B, C, N)
    in_maps = []
    for core in range(NC):
        b = core // 4
        g = core % 4
        sl = slice(GD * g, GD * (g + 1))
        w_all = np.concatenate(
            [
                wrep(Wq, g).reshape(C, 192),
                wrep(Wk, g).reshape(C, 192),
                Wv[sl, :].T,
            ],
            axis=1,
        )
        b4 = np.stack(
            [b3(bq, g, 0), b3(bq, g, 1), b3(bk, g, 0), b3(bk, g, 1)], axis=1
        )
        brep = np.concatenate(
            [bv[sl], bo if g == 0 else np.zeros(C, np.float32)]
        ).reshape(1, GD + C)
        in_maps.append(
            {
                "x": np.ascontiguousarray(xf[b].astype(NPBF16)),
                "w_all": np.ascontiguousarray(w_all.astype(NPBF16)),
                "b4": np.ascontiguousarray(b4),
                "brep": np.ascontiguousarray(brep),
                "wo_t": np.ascontiguousarray(Wo[:, sl].T.astype(NPBF16)),
            }
        )

    if _CACHED_NC is None:
        _CACHED_NC = build_nc()
    res = run_bass_kernel_spmd(_CACHED_NC, in_maps, core_ids=list(range(NC)))
    LAST_RESULTS = res

    out = np.zeros((B, C, N), dtype=np.float32)
    for core in range(NC):
        out[core // 4] += res.results[core]["out_t"].T
    return out.reshape(B, C, 48, 48)


# revision 14
# speedup vs baseline: 1.2321x; 1.2321x over previous
"""CrossSpectralAttention Trainium2 kernel (bf16, pipelined, v4).

Multi-head attention over 48x48 spatial tokens: B=2, C=256, 8 heads x
head_dim 32, N=2304 tokens. Sharded over 8 NeuronCores as 2 batches x 4
head-groups (2 heads per core). Each core computes its heads' Q/K/V
projections, attention, and a partial output projection (column slice of
Wo); the host sums the 4 partials per batch.

The wall-clock floor is the softmax exp: 2 heads x N^2 = 10.6M elements
through ScalarE at 1 elem/cycle/lane = ~85us busy. Everything else is
arranged to keep ScalarE saturated from ~12us (first score block) to the
end:

- Scores are 3-band row-tiled (q/k replicated 3x on 96 partitions); exp
  runs once per group on the full [128, 1536] PSUM block. The 256-wide
  tail q-piece uses 6-chunk groups -> same 1536-col call density.
  NOTE: EXP ACTIVATE with scale=1.0 is ~20% SLOWER than scale!=1.0 on
  this silicon (measured 1848ns vs 1540ns for identical [128,1536]
  calls), so the softmax 1/sqrt(d) scale stays in the instruction.
- PV is 2-band col-tiled (tile_position=(0,0)/(0,64)): chunk j
  accumulates into PSUM partitions 64*(j%2)+[0,33); two concurrent
  column-group matmuls double PV throughput. The drain is one DVE copy
  (band1 -> SBUF) + one DVE add (band0 + band1 -> oc bf16).
- Softmax denominators ride as a ones-column in vhat (row 32 of each
  band). Reciprocals are computed DENSELY: the den row [1, qln] bounces
  via DRAM into [128, qln/128], one tiny DVE reciprocal, back to DRAM,
  then a stride-0 partition-broadcast read. All 4 hops ride the gpsimd
  DMA queue (FIFO-ordered).
- V^T projection is batched 3 chunks per PSUM tile / one DVE drain, and
  streamed through the h0 attention slots (not a serial prologue).
- A dummy 4-element exp right after the input DMAs pulls the ~1.5us
  ACT_TABLE_LOAD into the DMA window.
- Leftover output-projection chunks after the last attention piece run
  in a dedicated double-buffered PSUM pool so their matmul->drain->DMA
  chains pipeline instead of serializing on one buffer.
"""

import numpy as np
import ml_dtypes

import concourse.bass as bass
import concourse.tile as tile
from concourse import mybir
from concourse.bass_utils import run_bass_kernel_spmd

B = 2
C = 256
N = 2304  # 48*48
NH = 8  # total heads
HPC = 2  # heads per core
HD = 32  # head dim
GD = HPC * HD  # 64 dims per core
NC = 8  # cores
NQB = 512  # query-block size for attention
NCH = N // 128  # 18 m-chunks
SCALE = float(HD) ** -0.5

F32 = mybir.dt.float32
BF16 = mybir.dt.bfloat16
NPBF16 = ml_dtypes.bfloat16

LAST_RESULTS = None  # BassKernelResults of the most recent run (for test.py)
_CACHED_NC = None


def _split_excess_waits(nc, max_waits=1):
    """This walrus build allows a single sync-wait per instruction; move
    excess waits onto same-engine NoOps inserted before the instruction."""
    state = {"uid": 0}

    def fix_block(b):
        i = 0
        insts = b.instructions
        while i < len(insts):
            inst = insts[i]
            for sub in getattr(inst, "blocks", None) or []:
                fix_block(sub)
            si = inst.sync_info
            if si is not None and si.on_wait and len(si.on_wait) > max_waits:
                waits = list(si.on_wait)
                keep, extra = waits[:max_waits], waits[max_waits:]
                inst.sync_info = mybir.SyncInfo(
                    on_wait=keep, on_update=list(si.on_update or [])
                )
                nops = []
                for j in range(0, len(extra), max_waits):
                    nop = mybir.InstNoOp(name=f"WSPLIT-{state['uid']}", ins=[], outs=[])
                    state["uid"] += 1
                    nop.engine = inst.engine
                    nop.sync_info = mybir.SyncInfo(
                        on_wait=extra[j : j + max_waits], on_update=[]
                    )
                    nops.append(nop)
                for k, nop in enumerate(nops):
                    insts.insert(i + k, nop)
                i += len(nops)
            i += 1

    for f in nc.m.functions:
        for b in f.blocks:
            fix_block(b)


def _pieces(total, piece):
    out = []
    o = 0
    while o < total:
        ln = min(piece, total - o)
        out.append((o, ln))
        o += ln
    return out


def build_nc(split=True):
    nc = bass.Bass()

    # wq_t/wk_t carry 3 replicated copies of each head's 32 W^T-columns so
    # the projection matmul writes the 3-band PE layout directly:
    # layout [C, 2 heads, 96] with cols (h, 32a+d) = W[32h+d, :].T
    # w_all packs wq|wk|wv column-wise: [0:192] q (h-major), [192:384] k,
    # [384:448] v - one DMA for all three projection weights.
    x_d = nc.dram_tensor("x", [C, N], BF16, kind="ExternalInput")
    wall_d = nc.dram_tensor("w_all", [C, 448], BF16, kind="ExternalInput")
    # b4 packs bq|bk per head column-wise: cols q0,q1,k0,k1 (3-band layout)
    b4_d = nc.dram_tensor("b4", [96, 4], F32, kind="ExternalInput")
    # brep packs bv (x3 replicas for batched V^T drains) | bo row-wise
    brep_d = nc.dram_tensor("brep", [1, 3 * GD + C], F32, kind="ExternalInput")
    wo_d = nc.dram_tensor("wo_t", [GD, C], BF16, kind="ExternalInput")
    out_d = nc.dram_tensor("out_t", [N, C], F32, kind="ExternalOutput")

    qpieces = _pieces(N, NQB)

    with tile.TileContext(nc) as tc:
        with (
            tc.tile_pool(name="singles", bufs=1) as singles,
            tc.tile_pool(name="expp", bufs=3) as expp,
            tc.tile_pool(name="invp", bufs=4) as invp,
            tc.tile_pool(name="ibcp", bufs=2) as ibcp,
            tc.tile_pool(name="obp", bufs=2) as obp,
            tc.tile_pool(name="outp", bufs=3) as outp,
        ):
            # ---- inputs to SBUF; first slab small so projections start early
            x_sb = singles.tile([128, 2, N], BF16)
            xr = x_d.rearrange("(c p) n -> p c n", p=128)
            w_sb = singles.tile([128, 2, 448], BF16)
            nc.gpsimd.dma_start(
                out=w_sb, in_=wall_d.rearrange("(c p) d -> p c d", p=128)
            )
            nc.sync.dma_start(out=x_sb[:, :, :512], in_=xr[:, :, :512])
            nc.sync.dma_start(out=x_sb[:, :, 512:1408], in_=xr[:, :, 512:1408])
            nc.sync.dma_start(out=x_sb[:, :, 1408:], in_=xr[:, :, 1408:])
            b4 = singles.tile([96, 4], F32)
            nc.gpsimd.dma_start(out=b4, in_=b4_d[:, :])
            # Wo^T for both heads, contracted in one matmul
            wo2 = singles.tile([GD, C], BF16)
            nc.scalar.dma_start(out=wo2, in_=wo_d[:, :])
            # bv (3 replicas) | bo replicated across partitions
            brep = singles.tile([128, 3 * GD + C], F32)
            nc.scalar.dma_start(
                out=brep,
                in_=bass.AP(
                    tensor=brep_d, offset=0, ap=[[0, 128], [1, 3 * GD + C]]
                ),
            )
            bv_rep3 = brep[:, : 3 * GD]
            bo_rep = brep[:, 3 * GD :]
            wof = {"q": 0, "k": 192}
            bcol = {("q", 0): 0, ("q", 1): 1, ("k", 0): 2, ("k", 1): 3}

            # dummy exp pulls the ACT table load into the DMA window
            dum = invp.tile([1, 4], F32, tag="dum", name="dum")
            nc.gpsimd.memset(dum, 0.0)
            nc.scalar.activation(
                out=dum, in_=dum, func=mybir.ActivationFunctionType.Exp,
                scale=SCALE,
            )

            # q/k in 3-band replicated layout [96, N] per head
            q_rep = [
                singles.tile([96, N], BF16, name=f"qrep{h}", tag=f"qrep{h}")
                for h in range(HPC)
            ]
            k_rep = [
                singles.tile([96, N], BF16, name=f"krep{h}", tag=f"krep{h}")
                for h in range(HPC)
            ]
            dest = {"q": q_rep, "k": k_rep}

            # vhat[:, j, h, :] = [V_t_h(chunk j) | 1] per head
            vhat = singles.tile([128, NCH, HPC, HD + 1], BF16)
            nc.gpsimd.memset(vhat[:, :, :, HD : HD + 1], 1.0)

            # oc[h] rows 0..31: head h's unnormalized output, row 32: its
            # softmax denominator. on2 rows 32h..32h+31: normalized.
            oc = [
                singles.tile([HD + 1, N], BF16, name=f"oc{h}", tag=f"oc{h}")
                for h in range(HPC)
            ]
            on2 = singles.tile([GD, N], BF16)

            def emit_proj(psum_pool, name, h, off, ln, tag="proj"):
                ps = psum_pool.tile([96, 512], F32, tag=tag, name="ps")
                for c in range(2):
                    nc.tensor.matmul(
                        ps[:, :ln],
                        w_sb[:, c, wof[name] + 96 * h : wof[name] + 96 * h + 96],
                        x_sb[:, c, off : off + ln],
                        start=(c == 0),
                        stop=(c == 1),
                    )
                nc.vector.tensor_scalar(
                    out=dest[name][h][:, off : off + ln],
                    in0=ps[:, :ln],
                    scalar1=b4[:, bcol[(name, h)] : bcol[(name, h)] + 1],
                    scalar2=None,
                    op0=mybir.AluOpType.add,
                )

            def emit_vt(psum_pool, b, tag="proj"):
                # V^T for chunks 3b..3b+2, one PSUM tile + one DVE drain
                tp = psum_pool.tile([128, 3 * GD], F32, tag=tag, name="tp")
                for jj in range(3):
                    j = 3 * b + jj
                    for c in range(2):
                        nc.tensor.matmul(
                            tp[:, GD * jj : GD * (jj + 1)],
                            x_sb[:, c, 128 * j : 128 * (j + 1)],
                            w_sb[:, c, 384:448],
                            start=(c == 0),
                            stop=(c == 1),
                        )
                nc.vector.tensor_tensor(
                    out=vhat[:, 3 * b : 3 * b + 3, :, :HD],
                    in0=tp.rearrange("p (j h d) -> p j h d", j=3, h=HPC),
                    in1=bv_rep3.rearrange("p (j h d) -> p j h d", j=3, h=HPC),
                    op=mybir.AluOpType.add,
                )

            # ---- upfront: k(h0) full, q(h0) piece0, V^T chunks 0..5 ----
            with tc.tile_pool(name="proj_psum", bufs=4, space="PSUM") as proj_psum:
                emit_proj(proj_psum, "k", 0, *qpieces[0][:2])
                emit_proj(proj_psum, "q", 0, *qpieces[0][:2])
                for off, ln in qpieces[1:]:
                    emit_proj(proj_psum, "k", 0, off, ln)
                emit_vt(proj_psum, 0)
                emit_vt(proj_psum, 1)

            # remaining projection work, streamed through the h0 attention
            # slots (one unit per group boundary). Ordering constraints:
            # Vt_b before PV group b of piece0; q0 piece p+1 within piece p;
            # k1/q1 before h1.
            units = [
                ("vt", 2), ("vt", 3), ("q", 0, 1), ("vt", 4), ("vt", 5),
                ("q", 0, 2), ("k", 1, 0), ("k", 1, 1), ("k", 1, 2), ("k", 1, 3),
                ("q", 0, 3), ("k", 1, 4), ("q", 1, 0), ("q", 1, 1), ("q", 1, 2),
                ("q", 0, 4), ("q", 1, 3), ("q", 1, 4),
            ]

            def emit_unit(u):
                if u[0] == "vt":
                    emit_vt(mixp, u[1], tag="mix")
                else:
                    name, h, p = u
                    emit_proj(mixp, name, h, *qpieces[p][:2], tag="mix")

            # ---- attention + normalize + output projection, pipelined ----
            with (
                tc.tile_pool(name="spsum", bufs=2, space="PSUM") as spsum,
                tc.tile_pool(name="opsum", bufs=1, space="PSUM") as opsum,
                tc.tile_pool(name="mixp", bufs=1, space="PSUM") as mixp,
                tc.tile_pool(name="dram", bufs=1, space="DRAM") as dramp,
            ):
                den_dram = dramp.tile([HPC, N], BF16, tag="dend")
                inv_dram = dramp.tile([HPC, N], F32, tag="invd")

                def emit_wo(j, pool):
                    wp = pool.tile([128, 512], F32, tag="mix", name="wp")
                    nc.tensor.matmul(
                        wp[:, :C],
                        on2[:, 128 * j : 128 * (j + 1)],
                        wo2,
                        start=True,
                        stop=True,
                    )
                    ot = outp.tile([128, C], F32, tag="ot")
                    nc.vector.tensor_tensor(
                        out=ot, in0=wp[:, :C], in1=bo_rep, op=mybir.AluOpType.add
                    )
                    nc.sync.dma_start(
                        out=out_d[128 * j : 128 * (j + 1), :], in_=ot
                    )

                def emit_pv(h, chunks, ex, o_ps, qln):
                    # chunk j accumulates into PV band j%2 (PSUM partitions
                    # 64*(j%2)+[0,33)); the two bands' column-group matmuls
                    # run concurrently on the PE.
                    for i, j in enumerate(chunks):
                        bb = j % 2
                        nc.tensor.matmul(
                            o_ps[64 * bb : 64 * bb + 33, :qln],
                            vhat[:, j, h, :],
                            ex[:, qln * i : qln * i + qln],
                            start=(j < 2),
                            stop=(j >= NCH - 2),
                            tile_position=(0, 64 * bb),
                        )

                pend = []  # output-projection chunks ready to emit
                for h in range(HPC):
                    for qoff, qln in qpieces:
                        j0 = qoff // 128
                        nj = qln // 128
                        # 512-wide pieces: 3 chunks/group (3 banks); 256-wide
                        # tail: 6 chunks/group - same 1536-col exp calls.
                        cpg = 6 if qln == 256 else 3
                        groups = [
                            list(range(g, min(g + cpg, NCH)))
                            for g in range(0, NCH, cpg)
                        ]
                        o_ps = opsum.tile([128, NQB], F32, tag="o")
                        ex_prev = None
                        chunks_prev = None
                        for gi, chunks in enumerate(groups):
                            s_tri = spsum.tile([128, 3 * NQB], F32, tag="s")
                            for i, j in enumerate(chunks):
                                # chunks sharing a PSUM bank must share a row
                                # band (serialize); distinct banks may differ.
                                a = (i % 3) if qln == NQB else ((i // 2) % 3)
                                nc.tensor.matmul(
                                    s_tri[:, qln * i : qln * i + qln],
                                    k_rep[h][
                                        32 * a : 32 * a + 32,
                                        128 * j : 128 * j + 128,
                                    ],
                                    q_rep[h][
                                        32 * a : 32 * a + 32, qoff : qoff + qln
                                    ],
                                    start=True,
                                    stop=True,
                                )
                            if gi > 0:
                                emit_pv(h, chunks_prev, ex_prev, o_ps, qln)
                            if gi >= 1:
                                if h == 0 and units:
                                    emit_unit(units.pop(0))
                                elif pend:
                                    emit_wo(pend.pop(0), mixp)
                            ex = expp.tile([128, 3 * NQB], BF16, tag="ex")
                            ncol = qln * len(chunks)
                            nc.scalar.activation(
                                out=ex[:, :ncol],
                                in_=s_tri[:, :ncol],
                                func=mybir.ActivationFunctionType.Exp,
                                scale=SCALE,
                            )
                            ex_prev = ex
                            chunks_prev = chunks
                        emit_pv(h, chunks_prev, ex_prev, o_ps, qln)
                        # band-reduce: DVE reads at most one PSUM operand per
                        # instruction, so stage band1 in SBUF then add band0.
                        # (row 32 = softmax denominator)
                        ob1 = obp.tile([33, NQB], BF16, tag="ob1")
                        nc.vector.tensor_copy(
                            out=ob1[:, :qln], in_=o_ps[64:97, :qln]
                        )
                        nc.vector.tensor_tensor(
                            out=oc[h][:, qoff : qoff + qln],
                            in0=o_ps[0:33, :qln],
                            in1=ob1[:, :qln],
                            op=mybir.AluOpType.add,
                        )
                        # dense reciprocal: den row -> DRAM -> [128, qln/128]
                        # -> 1/x -> DRAM -> stride-0 broadcast read [32, qln].
                        # All hops on the gpsimd DMA queue (FIFO order).
                        nc.gpsimd.dma_start(
                            out=den_dram[h : h + 1, qoff : qoff + qln],
                            in_=oc[h][HD : HD + 1, qoff : qoff + qln],
                        )
                        nd = qln // 128
                        dsrc = den_dram[h : h + 1, qoff : qoff + qln]
                        dent_bf = invp.tile(
                            [128, 4], BF16, tag="invb", name="dent_bf"
                        )
                        nc.gpsimd.dma_start(
                            out=dent_bf[:, :nd],
                            in_=bass.AP(
                                tensor=dsrc.tensor,
                                offset=dsrc.offset,
                                ap=[[nd, 128], [1, nd]],
                            ),
                        )
                        dinv = invp.tile([128, 4], F32, tag="invf", name="dinv")
                        nc.vector.tensor_copy(
                            out=dinv[:, :nd], in_=dent_bf[:, :nd]
                        )
                        nc.vector.reciprocal(out=dinv[:, :nd], in_=dinv[:, :nd])
                        idst = inv_dram[h : h + 1, qoff : qoff + qln]
                        nc.gpsimd.dma_start(
                            out=bass.AP(
                                tensor=idst.tensor,
                                offset=idst.offset,
                                ap=[[nd, 128], [1, nd]],
                            ),
                            in_=dinv[:, :nd],
                        )
                        ibc = ibcp.tile([HD, NQB], F32, tag="ibc")
                        src = inv_dram[h : h + 1, qoff : qoff + qln]
                        bc = bass.AP(
                            tensor=src.tensor,
                            offset=src.offset,
                            ap=[[0, HD]] + [list(d) for d in src.ap[1:]],
                        )
                        nc.gpsimd.dma_start(out=ibc[:, :qln], in_=bc)
                        nc.vector.tensor_tensor(
                            out=on2[HD * h : HD * (h + 1), qoff : qoff + qln],
                            in0=oc[h][:HD, qoff : qoff + qln],
                            in1=ibc[:, :qln],
                            op=mybir.AluOpType.mult,
                        )
                        if h == 1:
                            pend.extend(range(j0, j0 + nj))
            # leftover output-projection chunks: dedicated double-buffered
            # PSUM (spsum banks are free now) so the chains pipeline.
            with tc.tile_pool(name="wop", bufs=3, space="PSUM") as wop:
                while pend:
                    emit_wo(pend.pop(0), wop)

    if split:
        _split_excess_waits(nc)
    return nc


def kernel(x, Wq, bq, Wk, bk, Wv, bv, Wo, bo):
    global LAST_RESULTS, _CACHED_NC
    x = np.asarray(x, dtype=np.float32)
    Wq = np.asarray(Wq, dtype=np.float32)
    Wk = np.asarray(Wk, dtype=np.float32)
    Wv = np.asarray(Wv, dtype=np.float32)
    Wo = np.asarray(Wo, dtype=np.float32)
    bq = np.asarray(bq, dtype=np.float32)
    bk = np.asarray(bk, dtype=np.float32)
    bv = np.asarray(bv, dtype=np.float32)
    bo = np.asarray(bo, dtype=np.float32)

    def wrep(W, g):
        # [C, 2, 96]: head h cols = W[64g+32h : 64g+32h+32, :].T tiled 3x
        out = np.empty((C, HPC, 96), np.float32)
        for h in range(HPC):
            blk = W[GD * g + HD * h : GD * g + HD * (h + 1), :].T  # [C, 32]
            out[:, h, :] = np.tile(blk, (1, 3))
        return out

    def b3(bvec, g, h):
        return np.tile(bvec[GD * g + HD * h : GD * g + HD * (h + 1)], 3)

    xf = x.reshape(B, C, N)
    in_maps = []
    for core in range(NC):
        b = core // 4
        g = core % 4
        sl = slice(GD * g, GD * (g + 1))
        w_all = np.concatenate(
            [
                wrep(Wq, g).reshape(C, 192),
                wrep(Wk, g).reshape(C, 192),
                Wv[sl, :].T,
            ],
            axis=1,
        )
        b4 = np.stack(
            [b3(bq, g, 0), b3(bq, g, 1), b3(bk, g, 0), b3(bk, g, 1)], axis=1
        )
        # bv 3 replicas (for batched V^T drains) then bo
        brep = np.concatenate(
            [np.tile(bv[sl], 3), bo if g == 0 else np.zeros(C, np.float32)]
        ).reshape(1, 3 * GD + C)
        in_maps.append(
            {
                "x": np.ascontiguousarray(xf[b].astype(NPBF16)),
                "w_all": np.ascontiguousarray(w_all.astype(NPBF16)),
                "b4": np.ascontiguousarray(b4),
                "brep": np.ascontiguousarray(brep),
                "wo_t": np.ascontiguousarray(Wo[:, sl].T.astype(NPBF16)),
            }
        )

    if _CACHED_NC is None:
        _CACHED_NC = build_nc()
    res = run_bass_kernel_spmd(_CACHED_NC, in_maps, core_ids=list(range(NC)))
    LAST_RESULTS = res

    out = np.zeros((B, C, N), dtype=np.float32)
    for core in range(NC):
        out[core // 4] += res.results[core]["out_t"].T
    return out.reshape(B, C, 48, 48)
